# revision 39
# baseline (speedup 1.0000x reference)
"""Block-sparse (banded) attention kernel for Trainium2, 8 NeuronCores.

Sharding: data-parallel over batch (2) x tensor-parallel over heads
(16 heads -> 4 per core).  Each core computes its 4 heads' Q/K/V
projections, banded block attention (|r-c| <= 15 blocks, per-block
softmax), and a partial output projection; the host sums the 4 partial
outputs per batch element (bf16 partials, fp32 host accumulation).

Attention processes HEAD-PAIRS: head 2*hp sits in PE row-half 0-63 and
head 2*hp+1 in rows 64-127.  All attention matmuls use only two PE
configurations -- (64,128) for QK/broadcast at row offsets {0,64} and
(128,128) for denominator/AV -- so the PE pipeline never reconfigures
between unrelated shapes: zero-padded stationaries make the
denominator and AV write full 128-row PSUM banks where the two heads'
rows coexist (the padding rows contribute exact zeros to the
accumulation).  exp runs on Act, the normalization multiply on DVE;
GpSimd pre-zeroes SBUF staging (it has no PSUM port).

bf16 matmul pipeline, fp32 PSUM accumulation.
"""

import sys

for _p in ("/opt/trn_rl_repo",):
    if _p not in sys.path:
        sys.path.insert(0, _p)

from contextlib import ExitStack

import ml_dtypes
import numpy as np

import concourse.bacc as bacc
import concourse.tile as tile
from concourse import bass_utils, mybir

F32 = mybir.dt.float32
BF16 = mybir.dt.bfloat16
EXP = mybir.ActivationFunctionType.Exp
BF16NP = ml_dtypes.bfloat16

B, S, E = 2, 2048, 1024
H, HD, BLK = 16, 64, 64
NB = S // BLK  # 32 blocks
NCORES = 8
HPC = 4  # heads per core
F = HPC * HD  # 256 local features
BAND = 15
SCALE = HD ** -0.5

# per r8-slab (8 query blocks, q=512) column-block ranges, even-extended
T_SLABS = 4
QS = 512  # q extent per slab
LO = []
NP_T = []
for _t in range(T_SLABS):
    lo = max(0, 8 * _t - BAND)
    hi = min(NB - 1, 8 * _t + 7 + BAND)
    if (hi - lo + 1) % 2 == 1:
        if lo > 0:
            lo -= 1
        else:
            hi += 1
    LO.append(lo)
    NP_T.append((hi - lo + 1) // 2)
MAXP = max(NP_T)  # 16 pairs

AVLAG = 6  # slots between the normalization multiply and the AV matmul

# valid q-extent (block-aligned, contiguous) of each column-block pair within
# its slab: pair (c0, c0+1) only interacts with query blocks r in
# [c0-BAND, c0+1+BAND]
QLQH = []
for _t in range(T_SLABS):
    row = []
    for _j in range(NP_T[_t]):
        c0 = LO[_t] + 2 * _j
        rlo = max(8 * _t, c0 - BAND)
        rhi = min(8 * _t + 7, c0 + 1 + BAND)
        row.append(((rlo - 8 * _t) * 64, (rhi + 1 - 8 * _t) * 64))
    QLQH.append(row)


class _UP:
    """One head-pair unit: heads (2*hp, 2*hp+1) over slab t."""

    __slots__ = ("t", "hp", "npt", "lo", "ets", "accs", "rc", "acco", "pts")


def build_nc(debug=False):
    nc = bacc.Bacc("TRN2", target_bir_lowering=False, debug=False)

    xq_d = nc.dram_tensor("xqT", [E, S], BF16, kind="ExternalInput")
    xk_d = nc.dram_tensor("xkT", [E, S], BF16, kind="ExternalInput")
    xv_d = nc.dram_tensor("xvT", [E, S], BF16, kind="ExternalInput")
    wq_d = nc.dram_tensor("wqT", [E, F], BF16, kind="ExternalInput")
    wk_d = nc.dram_tensor("wkT", [E, F], BF16, kind="ExternalInput")
    wv_d = nc.dram_tensor("wvT", [E, F], BF16, kind="ExternalInput")
    wo_d = nc.dram_tensor("woT", [F, E], BF16, kind="ExternalInput")
    # selb: per (unit-half u, pair j) a [128,64] map: exp row k of pair j
    # sums into accs row 64*u + 2j + k//64 (zeros elsewhere)
    sel_d = nc.dram_tensor("selc", [128, 2 * MAXP * 64], BF16, kind="ExternalInput")
    # btd2: rows 0-63 fan accs rows (2j + p//64) of half 0 out to the 128
    # exp rows of pair j; rows 64-127 the same for half 1
    btd_d = nc.dram_tensor("btd", [128, MAXP * 128], BF16, kind="ExternalInput")
    vm_d = nc.dram_tensor(
        "vmask", [128, T_SLABS * QS], mybir.dt.uint8, kind="ExternalInput"
    )
    out_d = nc.dram_tensor("out", [S, E], BF16, kind="ExternalOutput")

    with tile.TileContext(nc) as tc, ExitStack() as ctx, nc.allow_low_precision(
        reason="bf16 matmul pipeline; fp32 PSUM accumulation"
    ):
        pers = ctx.enter_context(tc.tile_pool(name="pers", bufs=1))
        qT = pers.tile([128, 2 * S], BF16, tag="qT")
        kT = pers.tile([128, 2 * S], BF16, tag="kT")
        vv = pers.tile([128, 16 * F], BF16, tag="vv")
        wq = pers.tile([128, 8 * F], BF16, tag="wq")
        wk = pers.tile([128, 8 * F], BF16, tag="wk")
        wv = pers.tile([128, 8 * F], BF16, tag="wv")
        wo = pers.tile([128, 2 * E], BF16, tag="wo")
        selb = pers.tile([128, 2 * MAXP * 64], BF16, tag="selb")
        btd = pers.tile([128, MAXP * 128], BF16, tag="btd")
        vm = pers.tile([128, T_SLABS * QS], mybir.dt.uint8, tag="vm")

        # k-projection weights first, per-chunk so the first matmul only
        # waits on chunk 0
        for e in range(8):
            nc.sync.dma_start(
                wk[:, e * F : (e + 1) * F],
                wk_d.ap()[e * 128 : (e + 1) * 128, :],
            )

        # xva lives across phases 1-2; its chunk DMAs interleave with the
        # xk chunks so the v projection's inputs land during the k projection
        xvap = ctx.enter_context(tc.tile_pool(name="xva", bufs=1))
        xva = xvap.tile([128, 8 * 2048], BF16, tag="xva")

        # ---- phases 1+2: k projection (two fold passes over a 4-bank psK)
        # overlapped with the v projection (psV in the other 4 banks) ----
        xqp = ctx.enter_context(tc.tile_pool(name="xq", bufs=4))
        with tc.tile_pool(name="xk", bufs=1) as xkp, tc.tile_pool(
            name="psK", bufs=1, space="PSUM"
        ) as pskp, tc.tile_pool(name="psV", bufs=1, space="PSUM") as psvp:
            psK = pskp.tile([128, 2048], F32)
            xks = []
            for e in range(8):
                xt = xkp.tile([128, S], BF16, tag=f"xk{e}")
                nc.sync.dma_start(xt[:], xk_d.ap()[e * 128 : (e + 1) * 128, :])
                nc.sync.dma_start(
                    xva[:, e * 2048 : (e + 1) * 2048],
                    xv_d.ap()[e * 128 : (e + 1) * 128, :],
                )
                nc.sync.dma_start(
                    wv[:, e * F : (e + 1) * F],
                    wv_d.ap()[e * 128 : (e + 1) * 128, :],
                )
                xks.append(xt)
                for sc in range(4):
                    nc.tensor.matmul(
                        psK[:, sc * 512 : (sc + 1) * 512],
                        wk[:, e * F : e * F + 128],
                        xt[:, sc * 512 : (sc + 1) * 512],
                        start=(e == 0),
                        stop=(e == 7),
                    )
            for sc in range(4):
                nc.scalar.copy(
                    kT[:, sc * 512 : (sc + 1) * 512],
                    psK[:, sc * 512 : (sc + 1) * 512],
                )
            for e in range(8):
                for sc in range(4):
                    nc.tensor.matmul(
                        psK[:, sc * 512 : (sc + 1) * 512],
                        wk[:, e * F + 128 : e * F + 256],
                        xks[e][:, sc * 512 : (sc + 1) * 512],
                        start=(e == 0),
                        stop=(e == 7),
                    )
            for sc in range(4):
                nc.scalar.copy(
                    kT[:, S + sc * 512 : S + (sc + 1) * 512],
                    psK[:, sc * 512 : (sc + 1) * 512],
                )
            # weights/constants for the later phases
            for e in range(8):
                nc.sync.dma_start(
                    wq[:, e * F : (e + 1) * F],
                    wq_d.ap()[e * 128 : (e + 1) * 128, :],
                )
            # prefetch slab 0's q-projection inputs
            xq0 = []
            for e2 in range(4):
                xt = xqp.tile([128, 2, 512], BF16, tag="xq")
                nc.sync.dma_start(
                    xt[:],
                    xq_d.ap()[e2 * 256 : (e2 + 1) * 256, 0:512].rearrange(
                        "(two p) q -> p two q", p=128
                    ),
                )
                xq0.append(xt)
            nc.sync.dma_start(selb[:], sel_d.ap())
            nc.sync.dma_start(btd[:], btd_d.ap())
            nc.sync.dma_start(vm[:], vm_d.ap())
            nc.sync.dma_start(
                wo[:].rearrange("p (c e) -> p c e", c=2),
                wo_d.ap().rearrange("(c p) e -> p c e", p=128),
            )
            # v projection (vv layout [s-within-pair, (cp, h)])
            for sc in range(4):
                pvs = [
                    psvp.tile([128, 256], F32, name=f"pv{sub}", tag=f"psV{sub}")
                    for sub in range(4)
                ]
                for e in range(8):
                    for sub in range(4):
                        nc.tensor.matmul(
                            pvs[sub][:],
                            xva[
                                :,
                                e * 2048 + sc * 512 + sub * 128 : e * 2048
                                + sc * 512
                                + (sub + 1) * 128,
                            ],
                            wv[:, e * F : (e + 1) * F],
                            start=(e == 0),
                            stop=(e == 7),
                        )
                for sub in range(4):
                    nc.scalar.copy(
                        vv[:, sc * 1024 + sub * 256 : sc * 1024 + (sub + 1) * 256],
                        pvs[sub][:],
                    )

        # ---- phase 3: q projection + attention + output projection ----
        # PSUM budget (8 banks): "ps" tag 4x[128,512] (QK scores, 2-slot
        # depth per head-half; also outproj/qproj accumulators), "bt" tag
        # 2x[128,512] (broadcast), "L" tag 2x[128,512] (accs/acco).
        psp = ctx.enter_context(tc.tile_pool(name="psp", bufs=3, space="PSUM"))
        btp = ctx.enter_context(tc.tile_pool(name="btp", bufs=3, space="PSUM"))
        longp = ctx.enter_context(tc.tile_pool(name="psL", bufs=2, space="PSUM"))
        expp = ctx.enter_context(tc.tile_pool(name="expS", bufs=2))
        ptp = ctx.enter_context(tc.tile_pool(name="pt", bufs=16))
        rcpp = ctx.enter_context(tc.tile_pool(name="rcp", bufs=2))
        attp = ctx.enter_context(tc.tile_pool(name="att", bufs=3))
        outp = ctx.enter_context(tc.tile_pool(name="outsb", bufs=3))

        # pre-zero exp staging (full-width j=0 multiplies read stale cells;
        # they must be finite, never uninitialized NaN bit patterns)
        for w in range(2):
            for tg in ("e0", "e1"):
                et = expp.tile([128, MAXP * QS], BF16, name="et", tag=tg)
                nc.gpsimd.memset(et[:], 0.0)

        def qproj(sc4, pre=None):
            pqs = [
                psp.tile([128, 512], F32, name=f"pq{fold}", tag="ps")
                for fold in range(2)
            ]
            for e2 in range(4):
                if pre is not None:
                    xt = pre[e2]
                else:
                    xt = xqp.tile([128, 2, 512], BF16, tag="xq")
                    nc.sync.dma_start(
                        xt[:],
                        xq_d.ap()[
                            e2 * 256 : (e2 + 1) * 256, sc4 * 512 : (sc4 + 1) * 512
                        ].rearrange("(two p) q -> p two q", p=128),
                    )
                for half in range(2):
                    e = 2 * e2 + half
                    for fold in range(2):
                        nc.tensor.matmul(
                            pqs[fold][:],
                            wq[:, e * F + fold * 128 : e * F + fold * 128 + 128],
                            xt[:, half, :],
                            start=(e == 0),
                            stop=(e == 7),
                        )
            for fold in range(2):
                nc.scalar.copy(
                    qT[:, fold * S + sc4 * 512 : fold * S + (sc4 + 1) * 512],
                    pqs[fold][:],
                )

        def outproj(t, atts):
            # atts: [pair0_tile, pair1_tile], each [128, 512] bf16
            for sc2 in range(4):
                pos = [
                    psp.tile([128, 512], F32, name=f"po{eh}", tag="ps")
                    for eh in range(2)
                ]
                for kc in range(2):
                    for eh in range(2):
                        nc.tensor.matmul(
                            pos[eh][:],
                            atts[kc][:, sc2 * 128 : sc2 * 128 + 128],
                            wo[:, kc * E + eh * 512 : kc * E + eh * 512 + 512],
                            start=(kc == 0),
                            stop=(kc == 1),
                        )
                ob = outp.tile([128, 1024], BF16, tag="outsb")
                for eh in range(2):
                    if (sc2 + eh) % 2 == 0:
                        nc.vector.tensor_copy(
                            ob[:, eh * 512 : (eh + 1) * 512], pos[eh][:]
                        )
                    else:
                        nc.scalar.copy(
                            ob[:, eh * 512 : (eh + 1) * 512], pos[eh][:]
                        )
                row = (4 * t + sc2) * 128
                nc.sync.dma_start(out_d.ap()[row : row + 128, :], ob[:])

        units = []
        for t in range(T_SLABS):
            for hp in range(2):
                u = _UP()
                u.t, u.hp, u.npt, u.lo = t, hp, NP_T[t], LO[t]
                units.append(u)
        atts_by_t = {t: [] for t in range(T_SLABS)}

        def emit_qk(u, half, j, ps):
            c0 = u.lo + 2 * j
            bp = 64 * half
            ql, qh = QLQH[u.t][j]
            nc.tensor.matmul(
                ps[:, ql:qh],
                kT[bp : bp + 64, u.hp * S + c0 * 64 : u.hp * S + c0 * 64 + 128],
                qT[
                    bp : bp + 64,
                    u.hp * S + u.t * QS + ql : u.hp * S + u.t * QS + qh,
                ],
                start=True,
                stop=True,
            )

        def emit_exp(u, half, j, ps):
            ql, qh = QLQH[u.t][j]
            nc.scalar.activation(
                u.ets[half][:, j * QS + ql : j * QS + qh], ps[:, ql:qh], EXP
            )

        def emit_denom(u, half, j):
            # accs rows 64*half + {2j, 2j+1}; the two halves' matmuls write
            # disjoint column-half subarrays and run concurrently
            first = j == 0
            last = j == u.npt - 1
            ql, qh = (0, QS) if first else QLQH[u.t][j]
            nc.tensor.matmul(
                u.accs[64 * half : 64 * half + 64, ql:qh],
                selb[:, (half * MAXP + j) * 64 : (half * MAXP + j + 1) * 64],
                u.ets[half][:, j * QS + ql : j * QS + qh],
                start=first,
                stop=last,
                skip_group_check=True,
            )

        def emit_rcp(u):
            rs = rcpp.tile([128, 512], F32, tag="rs")
            rc = rcpp.tile([128, 512], BF16, tag="rc")
            nc.vector.reciprocal_approx_fast(rs[:], u.accs[:])
            nc.gpsimd.memset(rc[:], 0.0)
            nc.vector.copy_predicated(
                rc[:], vm[:, u.t * QS : (u.t + 1) * QS], rs[:]
            )
            u.rc = rc
            u.acco = longp.tile([128, 512], F32, name="acco", tag="L")

        def emit_bcast(u, half, j, bt):
            bp = 64 * half
            ql, qh = (0, QS) if j == 0 else QLQH[u.t][j]
            nc.tensor.matmul(
                bt[:, ql:qh],
                btd[bp : bp + 64, j * 128 : (j + 1) * 128],
                u.rc[bp : bp + 64, ql:qh],
                start=True,
                stop=True,
            )

        def emit_mul(u, half, j, bt):
            ql, qh = (0, QS) if j == 0 else QLQH[u.t][j]
            pt = ptp.tile([128, 512], BF16, tag="pt")
            nc.vector.tensor_mul(
                pt[:, ql:qh],
                u.ets[half][:, j * QS + ql : j * QS + qh],
                bt[:, ql:qh],
            )
            u.pts[half].append(pt)

        def emit_av(u, half, j):
            cp = u.lo // 2 + j
            h = 2 * u.hp + half
            ql, qh = (0, QS) if j == 0 else QLQH[u.t][j]
            nc.tensor.matmul(
                u.acco[64 * half : 64 * half + 64, ql:qh],
                vv[:, cp * F + h * 64 : cp * F + h * 64 + 64],
                u.pts[half][j][:, ql:qh],
                start=(j == 0),
                stop=(j == u.npt - 1),
                skip_group_check=True,
            )

        def finish(u):
            attn_t = attp.tile([128, 512], BF16, tag="att")
            nc.scalar.copy(attn_t[:], u.acco[:])
            atts_by_t[u.t].append(attn_t)
            if u.hp == 1:
                outproj(u.t, atts_by_t[u.t])

        qproj(0, pre=xq0)
        prev = None
        for u in units:
            u.ets = [
                expp.tile([128, MAXP * QS], BF16, name="et0", tag="e0"),
                expp.tile([128, MAXP * QS], BF16, name="et1", tag="e1"),
            ]
            u.accs = longp.tile([128, 512], F32, name="accs", tag="L")
            u.pts = [[], []]
            nd = 0  # denominator slots emitted so far for u
            if prev is not None:
                emit_rcp(prev)
            for j in range(max(u.npt, prev.npt if prev is not None else 0)):
                if prev is not None and j < prev.npt:
                    bts = []
                    for half in range(2):
                        bt = btp.tile([128, 512], F32, tag="btp")
                        emit_bcast(prev, half, j, bt)
                        bts.append(bt)
                    for half in range(2):
                        emit_mul(prev, half, j, bts[half])
                    ja = j - AVLAG
                    if 0 <= ja < prev.npt:
                        for half in range(2):
                            emit_av(prev, half, ja)
                if j >= 2 and nd < u.npt and nd == j - 2:
                    for half in range(2):
                        emit_denom(u, half, nd)
                    nd += 1
                if j < u.npt:
                    pss = [
                        psp.tile([128, 512], F32, name=f"ps{h}", tag="ps")
                        for h in range(2)
                    ]
                    for half in range(2):
                        emit_qk(u, half, j, pss[half])
                    for half in range(2):
                        emit_exp(u, half, j, pss[half])
            # drain: trailing denominators for u, AVs + finish for prev
            while nd < u.npt:
                for half in range(2):
                    emit_denom(u, half, nd)
                nd += 1
            if prev is not None:
                for j in range(max(prev.npt - AVLAG, 0), prev.npt):
                    for half in range(2):
                        emit_av(prev, half, j)
                finish(prev)
            if u.hp == 0 and u.t + 1 < T_SLABS:
                qproj(u.t + 1)
            prev = u

        # final unit: reciprocal, broadcast/mul/AV sweep, finish
        emit_rcp(prev)
        for j in range(prev.npt):
            bts = []
            for half in range(2):
                bt = btp.tile([128, 512], F32, tag="btp")
                emit_bcast(prev, half, j, bt)
                bts.append(bt)
            for half in range(2):
                emit_mul(prev, half, j, bts[half])
            if j >= AVLAG:
                for half in range(2):
                    emit_av(prev, half, j - AVLAG)
        for j in range(max(prev.npt - AVLAG, 0), prev.npt):
            for half in range(2):
                emit_av(prev, half, j)
        finish(prev)

    nc.compile()
    return nc


_NC_CACHE = []


def _get_nc():
    if not _NC_CACHE:
        _NC_CACHE.append(build_nc())
    return _NC_CACHE[0]


def _host_consts():
    # selb: chunk (u, j) maps exp row k -> accs-half row 2j + k//64
    selc = np.zeros((128, 2 * MAXP * 64), np.float32)
    for u in range(2):
        for j in range(MAXP):
            for k in range(128):
                selc[k, (u * MAXP + j) * 64 + 2 * j + k // 64] = 1.0
    # btd2: chunk j, rows 0-63: accs row (2j + p//64) -> exp row p of half 0;
    # rows 64-127 the same shifted for half 1
    btdm = np.zeros((128, MAXP * 128), np.float32)
    for u in range(2):
        for j in range(MAXP):
            for p in range(128):
                btdm[64 * u + 2 * j + p // 64, j * 128 + p] = 1.0
    # vmask over accs rows: row 64u + 2j + b is in-band where |r - c| <= BAND
    vmask = np.zeros((128, T_SLABS * QS), np.float32)
    for t in range(T_SLABS):
        for u in range(2):
            for j in range(NP_T[t]):
                for b in range(2):
                    c = LO[t] + 2 * j + b
                    for qb in range(QS // BLK):
                        r = 8 * t + qb
                        if abs(r - c) <= BAND:
                            vmask[
                                64 * u + 2 * j + b,
                                t * QS + qb * 64 : t * QS + (qb + 1) * 64,
                            ] = 1.0
    return selc, btdm, vmask


def _in_maps(query, key, value, Wq, Wk, Wv, Wo):
    selc, btdm, vmask = _host_consts()
    selc = selc.astype(BF16NP)
    btdm = btdm.astype(BF16NP)
    vmask = vmask.astype(np.uint8)
    in_maps = []
    for c in range(NCORES):
        b, g = divmod(c, HPC)
        fs = slice(F * g, F * (g + 1))
        in_maps.append(
            {
                "xqT": np.ascontiguousarray(query[b].T).astype(BF16NP),
                "xkT": np.ascontiguousarray(key[b].T).astype(BF16NP),
                "xvT": np.ascontiguousarray(value[b].T).astype(BF16NP),
                "wqT": np.ascontiguousarray((Wq[fs, :] * SCALE).T).astype(BF16NP),
                "wkT": np.ascontiguousarray(Wk[fs, :].T).astype(BF16NP),
                "wvT": np.ascontiguousarray(Wv[fs, :].T).astype(BF16NP),
                "woT": np.ascontiguousarray(Wo[:, fs].T).astype(BF16NP),
                "selc": selc,
                "btd": btdm,
                "vmask": vmask,
            }
        )
    return in_maps


def kernel(query, key, value, Wq, Wk, Wv, Wo):
    query = np.asarray(query, np.float32)
    key = np.asarray(key, np.float32)
    value = np.asarray(value, np.float32)
    Wq = np.asarray(Wq, np.float32)
    Wk = np.asarray(Wk, np.float32)
    Wv = np.asarray(Wv, np.float32)
    Wo = np.asarray(Wo, np.float32)

    nc = _get_nc()
    in_maps = _in_maps(query, key, value, Wq, Wk, Wv, Wo)
    res = bass_utils.run_bass_kernel_spmd(nc, in_maps, core_ids=list(range(NCORES)))
    out = np.zeros((B, S, E), np.float32)
    for c in range(NCORES):
        b = c // HPC
        out[b] += res.results[c]["out"].astype(np.float32)
    return out
